# revision 4
# baseline (speedup 1.0000x reference)
"""Trainium2 Bass kernel for C = tril(A @ B), A/B lower-triangular 4096x4096 fp32.

v5: k-major pass 0. The head of the kernel is DMA-descriptor-latency bound
(~1.6-2us per dma_start regardless of size, ~150-200 GB/s aggregate early),
so the old slot-major pass 0 starved the PE until ~16us. Now chunk cc of the
l=0 column feeds ALL slots t>=cc (k = 4cc..4cc+3) with per-slot 128KB A
pieces: the first 0.9MB of input unlocks 10k+ PE cycles across 8 PSUM banks,
the PE runs real work from ~10us, and the HAM clock-gate un-throttles on it.
A^T is repacked k-slice-major (slice cc = slots cc..7 x 4 k-blocks) which
also gives the early A loads wide (2-8KB) partition rows - good descriptor
economics. A few dummy matmuls cover the first data latency for HAM.

fp16 output (host converts back). Distribution unchanged: 8 cores =
4 row-groups x 2 col-groups; slot t of core (g,h) owns row-block 4t+g with
uniform K bound 4(t+1); local col l is global 512-col tile 2l+h with uniform
K start 8l; over-compute hits exact structural zeros.
"""

import numpy as np

N = 4096
P = 128
NCORES = 8
RG, CG = 4, 2           # row groups x col groups
SLOTS = N // P // RG    # 8 row-block slots per core
L = N // 512 // CG      # 4 local 512-col tiles per core
KB = N // P             # 32 k-blocks
CW = 512                # matmul free dim (fp32 max)
KC = 4                  # k-blocks per B chunk

MM_DT_NAME = "float16"  # float32 | float32r | float16 | bfloat16
DW = 256                # dummy matmul width
NHEAD = 10              # contiguous head warm-up dummies: end ~11.0us,
                        # matching the typical 10.9-11.6us first-data arrival
FILL_K0 = 3             # dummies right after the very first matmul
# fillers are only legal inside chunk 0 (all 8 PSUM banks are open
# accumulators from chunk 1 of pass 0 onward)
FILLERS = {(0, 0): 2, (0, 1): 1, (0, 2): 1, (0, 4): 1}

# A^T k-slice-major layout: slice cc holds slots t=cc..7, k-blocks 4cc..4cc+3
AS_KB = [(SLOTS - cc) * KC for cc in range(SLOTS)]     # k-blocks per slice
AS_OFF = [sum(AS_KB[:c]) for c in range(SLOTS)]
AT_TOT = sum(AS_KB)                                    # 144 k-blocks

# sub-splits of the big early slices so the first slots' pieces land in
# consumption order (unsplitting slice 1 measurably starves chunk 1 and
# re-throttles the HAM clock gate)
AS_SPLIT = {0: [(0, 2), (2, 5), (5, 8)], 1: [(1, 4), (4, 8)]}

B_CHUNKS = [(l, cc) for l in range(L) for cc in range((KB - 8 * l) // KC)]
B_CI = {(l, cc): i for i, (l, cc) in enumerate(B_CHUNKS)}

_cached = {}


def _build(mm_dt_name):
    import concourse.mybir as mybir
    import concourse.tile as tile
    from concourse import bacc

    mm_dt = getattr(mybir.dt, mm_dt_name)

    nc = bacc.Bacc("TRN2", target_bir_lowering=False, debug=False,
                   num_devices=NCORES)
    at_d = nc.dram_tensor("at", [P, AT_TOT * P], mm_dt,
                          kind="ExternalInput").ap()
    b_d = nc.dram_tensor("b", [len(B_CHUNKS) * P, KC * CW], mm_dt,
                         kind="ExternalInput").ap()
    o_d = nc.dram_tensor("o", [SLOTS, P, L * CW], mybir.dt.float16,
                         kind="ExternalOutput").ap()

    with tile.TileContext(nc) as tc:
        with (
            tc.tile_pool(name="atp", bufs=1) as atp,
            tc.tile_pool(name="bp", bufs=12) as bp,
            tc.tile_pool(name="b0p", bufs=1) as b0p,
            tc.tile_pool(name="zp", bufs=1) as zp,
            tc.tile_pool(name="pp", bufs=1, space="PSUM") as pp,
            tc.tile_pool(name="sp", bufs=3) as sp,
        ):
            atk = {}
            chunks = {}

            # warm-up scratch: dummies write slot 7's PSUM bank before its
            # real group opens
            zw = zp.tile([P, P + DW], mm_dt, tag="zw", name="zw")
            nc.vector.memzero(zw[:])

            def dummy_mms(n, tag):
                psw = pp.tile([P, DW], mybir.dt.float32, tag="ps7",
                              name=f"psw_{tag}")
                for i in range(n):
                    nc.tensor.matmul(psw[:], lhsT=zw[:, :P], rhs=zw[:, P:],
                                     start=True, stop=True)

            def load_slice(cc):
                a = atk[cc] = atp.tile([P, AS_KB[cc], P], mm_dt,
                                       tag=f"atk{cc}", name=f"atk{cc}")
                base = AS_OFF[cc] * P
                if cc in AS_SPLIT:
                    for (t0, t1) in AS_SPLIT[cc]:
                        j0, j1 = (t0 - cc) * KC * P, (t1 - cc) * KC * P
                        nc.scalar.dma_start(
                            a[:, (t0 - cc) * KC:(t1 - cc) * KC, :],
                            at_d[:, base + j0:base + j1])
                else:
                    nc.scalar.dma_start(a[:],
                                        at_d[:, base:base + AS_KB[cc] * P])

            def at_ap(t, k):
                cc = k // KC
                return atk[cc][:, (t - cc) * KC + k % KC, :]

            def load_chunk(l, cc):
                ci = B_CI[(l, cc)]
                bch = bp.tile([P, KC, CW], mm_dt, tag="b", name=f"b{ci}")
                nc.sync.dma_start(bch[:], b_d[ci * P:(ci + 1) * P, :])
                chunks[(l, cc)] = bch

            def evict(t, l, ps):
                st = sp.tile([P, CW], mybir.dt.float16, tag="st",
                             name=f"st{t}_{l}")
                nc.vector.tensor_copy(st[:], ps[:])
                if t == SLOTS - 1 and l == L - 1:
                    # very last eviction is on the critical exit path: split
                    # across both HWDGE queues to halve the descriptor-bound
                    # (1KB-row) transfer time
                    half = CW // 2
                    nc.sync.dma_start(o_d[t, :, l * CW:l * CW + half],
                                      st[:, :half])
                    nc.scalar.dma_start(o_d[t, :, l * CW + half:(l + 1) * CW],
                                        st[:, half:])
                    return
                # final pass: sync ring is done with B loads and has lower
                # first-byte latency than SWDGE - shortens the exit tail
                eng = nc.sync if l == L - 1 else nc.gpsimd
                eng.dma_start(o_d[t, :, l * CW:(l + 1) * CW], st[:])

            # first B chunk split so the k=0 tile (128 KB) lands first
            b00a = b0p.tile([P, 1, CW], mm_dt, tag="b0a", name="b00a")
            b00b = b0p.tile([P, KC - 1, CW], mm_dt, tag="b0b", name="b00b")
            nc.sync.dma_start(b00a[:], b_d[0:P, :CW])
            nc.sync.dma_start(b00b[:], b_d[0:P, CW:])

            dummy_mms(NHEAD, "head")

            def p0_rhs(cc, q, w):
                if cc == 0:
                    return b00a[:, 0, :w] if q == 0 else b00b[:, q - 1, :w]
                return chunks[(0, cc)][:, q, :w]

            # ---- pass 0 (l=0), k-major: chunk cc feeds slots t >= cc ----
            psums0 = {}
            for cc in range(SLOTS):
                load_slice(cc)
                if cc > 0:
                    load_chunk(0, cc)
                for t in range(cc, SLOTS):
                    if cc == 0:
                        psums0[t] = pp.tile([P, CW], mybir.dt.float32,
                                            tag=f"ps{t}", name=f"ps{t}_0")
                    for q in range(KC):
                        k = KC * cc + q
                        # head taper: at group k-offset d the col-tile is
                        # structurally zero beyond 128*(d+1) cols
                        w = min(CW, P * (k + 1))
                        nc.tensor.matmul(
                            psums0[t][:, :w], lhsT=at_ap(t, k),
                            rhs=p0_rhs(cc, q, w),
                            start=(k == 0), stop=(k == KC * (t + 1) - 1))
                        if cc == 0 and t == 0 and k == 0:
                            dummy_mms(FILL_K0, "fk0")
                    if t == cc:
                        evict(t, 0, psums0[t])
                    if (cc, t) in FILLERS:
                        dummy_mms(FILLERS[(cc, t)], f"f{cc}_{t}")

            # ---- passes 1..3: k-major ----
            # round-robin PSUM banks so a group never reopens the bank its
            # own pass just evicted (WAR stall on the DVE cast)
            bank_idx = SLOTS
            for l in range(1, L):
                psums = {}
                banks = {}
                for t in range(2 * l, SLOTS):
                    banks[t] = bank_idx % 8
                    bank_idx += 1
                for cc in range((KB - 8 * l) // KC):
                    load_chunk(l, cc)
                    for q in range(KC):
                        k = 8 * l + KC * cc + q
                        for t in range(2 * l, SLOTS):
                            kend = RG * (t + 1)
                            if k >= kend:
                                continue
                            if k == 8 * l:
                                psums[t] = pp.tile([P, CW], mybir.dt.float32,
                                                   tag=f"ps{banks[t]}",
                                                   name=f"ps{t}_{l}")
                            w = min(CW, P * (k - 8 * l + 1))
                            nc.tensor.matmul(
                                psums[t][:, :w],
                                lhsT=at_ap(t, k),
                                rhs=chunks[(l, cc)][:, q, :w],
                                start=(k == 8 * l),
                                stop=(k == kend - 1),
                            )
                            if k == kend - 1:
                                evict(t, l, psums[t])

    nc.compile()
    return nc


def _get_nc(mm_dt_name):
    if mm_dt_name not in _cached:
        _cached[mm_dt_name] = _build(mm_dt_name)
    return _cached[mm_dt_name]


def _np_dt(mm_dt_name):
    if mm_dt_name == "float16":
        return np.float16
    if mm_dt_name == "bfloat16":
        import ml_dtypes
        return ml_dtypes.bfloat16
    return np.float32


def _pack_b(B, h, np_dt=np.float32):
    """[20*128, 2048]: chunk (l, cc) row p = 4 k-tiles' (k = 8l+4cc ..) row p
    of global col-tile 2l+h, concatenated."""
    B = B.astype(np_dt)
    B4 = B.reshape(KB, P, N // CW, CW)
    slabs = []
    for l, cc in B_CHUNKS:
        ks = 8 * l + KC * cc
        slabs.append(
            B4[ks:ks + KC, :, 2 * l + h, :].transpose(1, 0, 2)
            .reshape(P, KC * CW))
    return np.ascontiguousarray(np.stack(slabs)).reshape(len(B_CHUNKS) * P,
                                                         KC * CW)


def _pack_at(A, g, np_dt=np.float32):
    """[128, 144*128] k-slice-major: slice cc = slots t=cc..7, k-blocks
    4cc..4cc+3 of row-block 4t+g, laid out (p, (t,kk), m)."""
    A = A.astype(np_dt)
    out = np.empty((P, AT_TOT * P), dtype=np_dt)
    for cc in range(SLOTS):
        pos = AS_OFF[cc] * P
        for t in range(cc, SLOTS):
            blk = RG * t + g
            ks = KC * cc * P
            blockT = A[blk * P:(blk + 1) * P, ks:ks + KC * P].T   # [kk, m]
            arr = blockT.reshape(KC, P, P).transpose(1, 0, 2)     # [p, kk, m]
            out[:, pos:pos + KC * P] = arr.reshape(P, KC * P)
            pos += KC * P
    return out


def kernel(A, B, mm_dt_name=MM_DT_NAME, trace=False):
    from concourse.bass_utils import run_bass_kernel_spmd

    A = np.ascontiguousarray(np.asarray(A, dtype=np.float32))
    B = np.ascontiguousarray(np.asarray(B, dtype=np.float32))

    nc = _get_nc(mm_dt_name)
    np_dt = _np_dt(mm_dt_name)
    b_packs = [_pack_b(B, h, np_dt) for h in range(CG)]
    in_maps = [{"at": _pack_at(A, c % RG, np_dt), "b": b_packs[c // RG]}
               for c in range(NCORES)]

    res = None
    for attempt in range(3):
        try:
            res = run_bass_kernel_spmd(nc, in_maps,
                                       core_ids=list(range(NCORES)),
                                       trace=trace)
            break
        except Exception:
            if attempt == 2:
                raise
            import time
            time.sleep(2)
    C = np.zeros((N, N), dtype=np.float32)
    for c in range(NCORES):
        g, h = c % RG, c // RG
        o = res.results[c]["o"].astype(np.float32)
        for t in range(SLOTS):
            blk = RG * t + g
            for l in range(L):
                jt = 2 * l + h
                C[blk * P:(blk + 1) * P, jt * CW:(jt + 1) * CW] = \
                    o[t, :, l * CW:(l + 1) * CW]
    if trace:
        kernel.last_exec_time_ns = res.exec_time_ns
        kernel.last_results = res
    return C
